# revision 6
# baseline (speedup 1.0000x reference)
"""DeepseekV3 MoE kernel for 8 Trainium2 NeuronCores.

Sharding: expert-parallel (2 routed experts per core) + intermediate-sharded
shared expert (128 of 1024 columns per core); gate replicated; per-chunk
ReduceScatter combines partial outputs; host concatenates the shards.

Self-contained: hardcodes all shapes. Only dependency is the concourse
tree (on PYTHONPATH in the container) and numpy.
"""

import os
import sys

import numpy as np

for _p in ("/opt/trn_rl_repo", "/root/.axon_site/_ro/trn_rl_repo"):
    if os.path.isdir(_p) and _p not in sys.path:
        sys.path.append(_p)

import concourse.bass as bass
import concourse.bacc as bacc
import concourse.mybir as mybir
import concourse.tile as tile
from concourse.bass_utils import run_bass_kernel_spmd
from concourse.masks import make_identity

F32 = mybir.dt.float32
F32R = mybir.dt.float32r
AX = mybir.AxisListType.X
OP = mybir.AluOpType
ACT = mybir.ActivationFunctionType

H = 1024          # hidden size
M = 512           # expert intermediate
E = 16            # routed experts
EPC = 2           # experts per core
NCORES = 8
N = 2048          # tokens (B*S)
CH = 512          # token chunk (free dim per matmul)
NCH = N // CH     # 4 chunks
KT = H // 128     # 8 contraction tiles
MB = M // 128     # 4 m-tiles per expert
HT = H // 128     # 8 output h-tiles
SCALE = 2.5
SM = 128          # shared-expert intermediate columns per core


def _routing(nc, pool, s4c, comb):
    """Token-major DeepseekV3 noaux_tc routing for one [128, 16] tile.

    s4c: sigmoid(logits) + bias, [128, 16] fp32 SBUF.
    comb: output combine weights [128, 16] (SCALE * topk_weight scattered).
    """
    v = s4c.rearrange("p (g s) -> p g s", g=4)

    # sum of top-2 per group of 4 = max over the 6 pairwise sums
    pairs = pool.tile([128, 24], F32, tag="rt_pairs")
    pv = pairs.rearrange("p (g s) -> p g s", g=4)
    nc.vector.tensor_add(pv[:, :, 0:3], v[:, :, 0:3], v[:, :, 1:4])
    nc.vector.tensor_add(pv[:, :, 3:5], v[:, :, 0:2], v[:, :, 2:4])
    nc.vector.tensor_add(pv[:, :, 5:6], v[:, :, 0:1], v[:, :, 3:4])
    gsum = pool.tile([128, 4], F32, tag="rt_gsum")
    nc.vector.reduce_max(out=gsum, in_=pv, axis=AX)

    # 2nd largest group sum = max over the 6 pairwise mins
    gmins = pool.tile([128, 8], F32, tag="rt_gmins")
    nc.vector.tensor_tensor(gmins[:, 0:3], gsum[:, 0:3], gsum[:, 1:4], op=OP.min)
    nc.vector.tensor_tensor(gmins[:, 3:5], gsum[:, 0:2], gsum[:, 2:4], op=OP.min)
    nc.vector.tensor_tensor(gmins[:, 5:6], gsum[:, 0:1], gsum[:, 3:4], op=OP.min)
    t2g = pool.tile([128, 1], F32, tag="rt_t2g")
    nc.vector.reduce_max(out=t2g, in_=gmins[:, 0:6], axis=AX)

    # group mask (1.0 for the top-2 groups), expanded to 16 experts
    gmask = pool.tile([128, 4], F32, tag="rt_gmask")
    nc.vector.tensor_scalar(gmask, gsum, t2g, None, op0=OP.is_ge)
    mask16 = pool.tile([128, 16], F32, tag="rt_mask16")
    m16v = mask16.rearrange("p (g s) -> p g s", g=4)
    for j in range(4):
        nc.vector.tensor_copy(m16v[:, :, j], gmask)

    masked = pool.tile([128, 16], F32, tag="rt_masked")
    nc.vector.tensor_mul(masked, s4c, mask16)

    # top-4 of 16 via Max8, threshold select, normalize
    top8 = pool.tile([128, 8], F32, tag="rt_top8")
    nc.vector.max(out=top8, in_=masked)
    denom = pool.tile([128, 1], F32, tag="rt_denom")
    nc.vector.reduce_sum(out=denom, in_=top8[:, 0:4], axis=AX)
    w = pool.tile([128, 1], F32, tag="rt_w")
    nc.vector.tensor_scalar_add(denom, denom, 1e-20)
    nc.vector.reciprocal(w, denom)
    nc.vector.tensor_scalar_mul(w, w, SCALE)

    sel = pool.tile([128, 16], F32, tag="rt_sel")
    nc.vector.tensor_scalar(sel, masked, top8[:, 3:4], None, op0=OP.is_ge)
    nc.vector.tensor_mul(sel, sel, masked)        # selected scores
    nc.vector.tensor_scalar_mul(comb, sel, w)     # scale / denom


def build_program():
    nc = bacc.Bacc(
        "TRN2",
        target_bir_lowering=False,
        debug=False,
        enable_asserts=False,
        num_devices=NCORES,
    )

    xT = nc.dram_tensor("xT", [H, N], F32R, kind="ExternalInput").ap()
    xTf = nc.dram_tensor("xTf", [H, N], F32, kind="ExternalInput").ap()
    gk = nc.dram_tensor("gk", [H, E], F32, kind="ExternalInput").ap()
    gbr = nc.dram_tensor("gbr", [128, E], F32, kind="ExternalInput").ap()
    wg = nc.dram_tensor("wg", [EPC, H, M], F32R, kind="ExternalInput").ap()
    wu = nc.dram_tensor("wu", [EPC, H, M], F32R, kind="ExternalInput").ap()
    wd = nc.dram_tensor("wd", [EPC, M, H], F32R, kind="ExternalInput").ap()
    sg = nc.dram_tensor("sg", [H, SM], F32R, kind="ExternalInput").ap()
    su = nc.dram_tensor("su", [H, SM], F32R, kind="ExternalInput").ap()
    sd = nc.dram_tensor("sd", [SM, H], F32R, kind="ExternalInput").ap()
    sel_in = nc.dram_tensor("sel", [EPC, E, 128], F32R, kind="ExternalInput").ap()
    out = nc.dram_tensor("out", [128, N], F32, kind="ExternalOutput").ap()

    with tile.TileContext(nc) as tc:
        with (
            tc.tile_pool(name="w", bufs=1) as wpool,
            tc.tile_pool(name="sb", bufs=2) as sb,
            tc.tile_pool(name="rt", bufs=2) as rt,
            tc.tile_pool(name="ps", bufs=2, space="PSUM") as ps,
            tc.tile_pool(name="dram", bufs=1, space="DRAM") as dram,
        ):
            # ---- resident weights ----
            wg_sb = []
            wu_sb = []
            wd_sb = []
            for e in range(EPC):
                g_t = wpool.tile([128, KT * M], F32R, name=f"wg_sb{e}", tag=f"wg{e}")
                u_t = wpool.tile([128, KT * M], F32R, name=f"wu_sb{e}", tag=f"wu{e}")
                d_t = wpool.tile([128, MB * H], F32R, name=f"wd_sb{e}", tag=f"wd{e}")
                for k in range(KT):
                    nc.sync.dma_start(
                        out=g_t[:, k * M:(k + 1) * M],
                        in_=wg[e, k * 128:(k + 1) * 128, :],
                    )
                    nc.sync.dma_start(
                        out=u_t[:, k * M:(k + 1) * M],
                        in_=wu[e, k * 128:(k + 1) * 128, :],
                    )
                for mb in range(MB):
                    nc.sync.dma_start(
                        out=d_t[:, mb * H:(mb + 1) * H],
                        in_=wd[e, mb * 128:(mb + 1) * 128, :],
                    )
                wg_sb.append(g_t)
                wu_sb.append(u_t)
                wd_sb.append(d_t)

            sg_sb = wpool.tile([128, KT * SM], F32R, tag="sg")
            su_sb = wpool.tile([128, KT * SM], F32R, tag="su")
            for k in range(KT):
                nc.sync.dma_start(
                    out=sg_sb[:, k * SM:(k + 1) * SM],
                    in_=sg[k * 128:(k + 1) * 128, :],
                )
                nc.sync.dma_start(
                    out=su_sb[:, k * SM:(k + 1) * SM],
                    in_=su[k * 128:(k + 1) * 128, :],
                )
            sd_sb = wpool.tile([128, H], F32R, tag="sd")
            nc.sync.dma_start(out=sd_sb, in_=sd)

            gk_sb = wpool.tile([128, KT * E], F32, tag="gk")
            for k in range(KT):
                nc.sync.dma_start(
                    out=gk_sb[:, k * E:(k + 1) * E],
                    in_=gk[k * 128:(k + 1) * 128, :],
                )
            gbr_sb = wpool.tile([128, E], F32, tag="gbr")
            nc.sync.dma_start(out=gbr_sb, in_=gbr)
            selm_sb = wpool.tile([E, EPC * 128], F32R, tag="selm")
            for e in range(EPC):
                nc.sync.dma_start(
                    out=selm_sb[:, e * 128:(e + 1) * 128], in_=sel_in[e]
                )

            ident = wpool.tile([128, 128], F32, tag="ident")
            make_identity(nc, ident)

            # ---- per-chunk pipeline ----
            for c in range(NCH):
                # activations chunk, feature-major [h, tok]
                xt = sb.tile([128, KT * CH], F32R, tag="xt")
                xtf = sb.tile([128, KT * CH], F32, tag="xtf", bufs=1)
                for k in range(KT):
                    nc.sync.dma_start(
                        out=xt[:, k * CH:(k + 1) * CH],
                        in_=xT[k * 128:(k + 1) * 128, c * CH:(c + 1) * CH],
                    )
                    nc.sync.dma_start(
                        out=xtf[:, k * CH:(k + 1) * CH],
                        in_=xTf[k * 128:(k + 1) * 128, c * CH:(c + 1) * CH],
                    )

                # gate: token-major fp32 logits (selection must be exact),
                # then routing per 128-token tile
                combT = sb.tile([E, CH], F32R, tag="combT")
                for t in range(CH // 128):
                    plt = ps.tile([128, E], F32, tag="pmisc")
                    for k in range(KT):
                        nc.tensor.matmul(
                            plt,
                            lhsT=xtf[:, k * CH + t * 128: k * CH + (t + 1) * 128],
                            rhs=gk_sb[:, k * E:(k + 1) * E],
                            start=(k == 0),
                            stop=(k == KT - 1),
                        )
                    s4c = rt.tile([128, E], F32, tag="rt_s4c")
                    nc.scalar.activation(s4c, plt, ACT.Sigmoid)
                    nc.vector.tensor_add(s4c, s4c, gbr_sb)
                    comb = rt.tile([128, E], F32, tag="rt_comb")
                    _routing(nc, rt, s4c, comb)
                    pct = ps.tile([E, 128], F32, tag="pmisc")
                    nc.tensor.transpose(pct, comb, ident)
                    nc.scalar.copy(combT[:, t * 128:(t + 1) * 128], pct)

                # broadcast each local expert's combine row across partitions
                cbc = []
                for e in range(EPC):
                    pb = ps.tile([128, CH], F32, tag="pmisc")
                    nc.tensor.matmul(
                        pb,
                        lhsT=selm_sb[:, e * 128:(e + 1) * 128],
                        rhs=combT,
                        start=True,
                        stop=True,
                    )
                    cb = sb.tile([128, CH], F32, tag=f"cbc{e}")
                    nc.scalar.copy(cb, pb)
                    cbc.append(cb)

                # routed experts: g/u projections, inter = silu(g)*u*combine
                inter = []
                for e in range(EPC):
                    it = sb.tile([128, MB * CH], F32R, tag=f"inter{e}", bufs=1)
                    for mb in range(MB):
                        pg = ps.tile([128, CH], F32, tag="pg")
                        for k in range(KT):
                            nc.tensor.matmul(
                                pg,
                                lhsT=wg_sb[e][:, k * M + mb * 128: k * M + (mb + 1) * 128],
                                rhs=xt[:, k * CH:(k + 1) * CH],
                                start=(k == 0),
                                stop=(k == KT - 1),
                            )
                        pu = ps.tile([128, CH], F32, tag="pu")
                        for k in range(KT):
                            nc.tensor.matmul(
                                pu,
                                lhsT=wu_sb[e][:, k * M + mb * 128: k * M + (mb + 1) * 128],
                                rhs=xt[:, k * CH:(k + 1) * CH],
                                start=(k == 0),
                                stop=(k == KT - 1),
                            )
                        sig_t = sb.tile([128, CH], F32, tag="sig")
                        nc.scalar.activation(sig_t, pg, ACT.Sigmoid)
                        sg_t = sb.tile([128, CH], F32, tag="silu")
                        # silu(g) = g * sigmoid(g)
                        nc.vector.scalar_tensor_tensor(
                            sg_t, pg, 1.0, sig_t, op0=OP.mult, op1=OP.mult
                        )
                        us = sb.tile([128, CH], F32, tag="us")
                        nc.vector.tensor_mul(us, pu, cbc[e])
                        nc.vector.tensor_mul(
                            it[:, mb * CH:(mb + 1) * CH], sg_t, us
                        )
                    inter.append(it)

                # shared expert slice (no combine scaling)
                pgs = ps.tile([128, CH], F32, tag="pg")
                for k in range(KT):
                    nc.tensor.matmul(
                        pgs,
                        lhsT=sg_sb[:, k * SM:(k + 1) * SM],
                        rhs=xt[:, k * CH:(k + 1) * CH],
                        start=(k == 0),
                        stop=(k == KT - 1),
                    )
                pus = ps.tile([128, CH], F32, tag="pu")
                for k in range(KT):
                    nc.tensor.matmul(
                        pus,
                        lhsT=su_sb[:, k * SM:(k + 1) * SM],
                        rhs=xt[:, k * CH:(k + 1) * CH],
                        start=(k == 0),
                        stop=(k == KT - 1),
                    )
                sig_s = sb.tile([128, CH], F32, tag="sig")
                nc.scalar.activation(sig_s, pgs, ACT.Sigmoid)
                sgs = sb.tile([128, CH], F32, tag="silu")
                nc.vector.scalar_tensor_tensor(
                    sgs, pgs, 1.0, sig_s, op0=OP.mult, op1=OP.mult
                )
                inter_s = sb.tile([128, CH], F32R, tag="inter_s")
                nc.vector.tensor_mul(inter_s, sgs, pus)

                # down projections, all experts + shared accumulated in PSUM
                ypart = dram.tile([H, CH], F32, name=f"ypart{c}")
                for ht in range(HT):
                    py = ps.tile([128, CH], F32, tag="py")
                    first = True
                    for e in range(EPC):
                        for mb in range(MB):
                            nc.tensor.matmul(
                                py,
                                lhsT=wd_sb[e][:, mb * H + ht * 128: mb * H + (ht + 1) * 128],
                                rhs=inter[e][:, mb * CH:(mb + 1) * CH],
                                start=first,
                                stop=False,
                            )
                            first = False
                    nc.tensor.matmul(
                        py,
                        lhsT=sd_sb[:, ht * 128:(ht + 1) * 128],
                        rhs=inter_s,
                        start=False,
                        stop=True,
                    )
                    yp = sb.tile([128, CH], F32, tag="yp")
                    nc.scalar.copy(yp, py)
                    nc.sync.dma_start(
                        out=ypart[ht * 128:(ht + 1) * 128, :], in_=yp
                    )

                # reduce across cores; each rank keeps its 128-row h shard
                rs_out = dram.tile([128, CH], F32, name=f"rsout{c}")
                nc.gpsimd.collective_compute(
                    "ReduceScatter",
                    OP.add,
                    replica_groups=[list(range(NCORES))],
                    ins=[ypart.opt()],
                    outs=[rs_out.opt()],
                )
                nc.sync.dma_start(out=out[:, c * CH:(c + 1) * CH], in_=rs_out)

    nc.compile()
    return nc


_NC_CACHE = None


def _get_program():
    global _NC_CACHE
    if _NC_CACHE is None:
        _NC_CACHE = build_program()
    return _NC_CACHE


def _make_in_maps(inputs):
    x = np.ascontiguousarray(
        np.asarray(inputs["hidden_states"], dtype=np.float32).reshape(N, H).T
    )
    gk = np.ascontiguousarray(np.asarray(inputs["gate_kernel"], dtype=np.float32))
    gb = np.asarray(inputs["gate_bias"], dtype=np.float32)
    gbr = np.ascontiguousarray(np.broadcast_to(gb[None, :], (128, E)))
    w_gate = np.asarray(inputs["w_gate"], dtype=np.float32)
    w_up = np.asarray(inputs["w_up"], dtype=np.float32)
    w_down = np.asarray(inputs["w_down"], dtype=np.float32)
    sw_gate = np.asarray(inputs["sw_gate"], dtype=np.float32)
    sw_up = np.asarray(inputs["sw_up"], dtype=np.float32)
    sw_down = np.asarray(inputs["sw_down"], dtype=np.float32)

    in_maps = []
    for c in range(NCORES):
        sel = np.zeros((EPC, E, 128), dtype=np.float32)
        for e in range(EPC):
            sel[e, EPC * c + e, :] = 1.0
        in_maps.append({
            "xT": x,
            "xTf": x,
            "gk": gk,
            "gbr": gbr,
            "wg": np.ascontiguousarray(w_gate[EPC * c:EPC * (c + 1)]),
            "wu": np.ascontiguousarray(w_up[EPC * c:EPC * (c + 1)]),
            "wd": np.ascontiguousarray(w_down[EPC * c:EPC * (c + 1)]),
            "sg": np.ascontiguousarray(sw_gate[:, SM * c:SM * (c + 1)]),
            "su": np.ascontiguousarray(sw_up[:, SM * c:SM * (c + 1)]),
            "sd": np.ascontiguousarray(sw_down[SM * c:SM * (c + 1), :]),
            "sel": sel,
        })
    return in_maps


def run(inputs, trace=False):
    """Returns (output, BassKernelResults)."""
    nc = _get_program()
    in_maps = _make_in_maps(inputs)
    res = run_bass_kernel_spmd(
        nc, in_maps, core_ids=list(range(NCORES)), trace=trace
    )
    yT = np.concatenate(
        [res.results[c]["out"] for c in range(NCORES)], axis=0
    )
    y = np.ascontiguousarray(yT.T).reshape(2, 1024, H).astype(np.float32)
    return y, res


def kernel(**inputs):
    y, _ = run(inputs, trace=False)
    return y


# revision 7
# speedup vs baseline: 1.1521x; 1.1521x over previous
"""DeepseekV3 MoE kernel for 8 Trainium2 NeuronCores.

Sharding: expert-parallel (2 routed experts per core) + intermediate-sharded
shared expert (128 of 1024 columns per core); gate replicated (computed in
full fp32 on every core, pipelined one chunk ahead of the expert compute);
per-chunk ReduceScatter combines partial outputs; host concatenates shards.

Self-contained: hardcodes all shapes. Only dependency is the concourse
tree (on PYTHONPATH in the container) and numpy.
"""

import os
import sys

import numpy as np

for _p in ("/opt/trn_rl_repo", "/root/.axon_site/_ro/trn_rl_repo"):
    if os.path.isdir(_p) and _p not in sys.path:
        sys.path.append(_p)

import concourse.bacc as bacc
import concourse.mybir as mybir
import concourse.tile as tile
from concourse.bass_utils import run_bass_kernel_spmd
from concourse.masks import make_identity

F32 = mybir.dt.float32
F32R = mybir.dt.float32r
AX = mybir.AxisListType.X
OP = mybir.AluOpType
ACT = mybir.ActivationFunctionType

H = 1024          # hidden size
M = 512           # expert intermediate
E = 16            # routed experts
EPC = 2           # experts per core
NCORES = 8
N = 2048          # tokens (B*S)
KT = H // 128     # 8 contraction tiles
MB = M // 128     # 4 m-tiles per expert
HT = H // 128     # 8 output h-tiles
SCALE = 2.5
SM = 128          # shared-expert intermediate columns per core

# token chunks: smaller final chunks shrink the un-overlapped tail
# (last ReduceScatter + output DMA)
CHW = [512, 512, 512, 256, 256]
CHOFF = [0, 512, 1024, 1536, 1792]
NCH = len(CHW)


def _routing(nc, pool, s4c, comb):
    """Token-major DeepseekV3 noaux_tc routing for one [128, 16] tile.

    s4c: sigmoid(logits) + bias, [128, 16] fp32 SBUF.
    comb: output combine weights [128, 16] (SCALE * topk_weight scattered).
    """
    v = s4c.rearrange("p (g s) -> p g s", g=4)

    # sum of top-2 per group of 4 = max over the 6 pairwise sums
    pairs = pool.tile([128, 24], F32, tag="rt_pairs")
    pv = pairs.rearrange("p (g s) -> p g s", g=4)
    nc.vector.tensor_add(pv[:, :, 0:3], v[:, :, 0:3], v[:, :, 1:4])
    nc.vector.tensor_add(pv[:, :, 3:5], v[:, :, 0:2], v[:, :, 2:4])
    nc.vector.tensor_add(pv[:, :, 5:6], v[:, :, 0:1], v[:, :, 3:4])
    gsum = pool.tile([128, 4], F32, tag="rt_gsum")
    nc.vector.reduce_max(out=gsum, in_=pv, axis=AX)

    # 2nd largest group sum = max over the 6 pairwise mins
    gmins = pool.tile([128, 8], F32, tag="rt_gmins")
    nc.vector.tensor_tensor(gmins[:, 0:3], gsum[:, 0:3], gsum[:, 1:4], op=OP.min)
    nc.vector.tensor_tensor(gmins[:, 3:5], gsum[:, 0:2], gsum[:, 2:4], op=OP.min)
    nc.vector.tensor_tensor(gmins[:, 5:6], gsum[:, 0:1], gsum[:, 3:4], op=OP.min)
    t2g = pool.tile([128, 1], F32, tag="rt_t2g")
    nc.vector.reduce_max(out=t2g, in_=gmins[:, 0:6], axis=AX)

    # group mask (1.0 for the top-2 groups), expanded to 16 experts
    gmask = pool.tile([128, 4], F32, tag="rt_gmask")
    nc.vector.tensor_scalar(gmask, gsum, t2g, None, op0=OP.is_ge)
    mask16 = pool.tile([128, 16], F32, tag="rt_mask16")
    m16v = mask16.rearrange("p (g s) -> p g s", g=4)
    for j in range(4):
        nc.vector.tensor_copy(m16v[:, :, j], gmask)

    masked = pool.tile([128, 16], F32, tag="rt_masked")
    nc.vector.tensor_mul(masked, s4c, mask16)

    # top-4 of 16 via Max8, threshold select, normalize
    top8 = pool.tile([128, 8], F32, tag="rt_top8")
    nc.vector.max(out=top8, in_=masked)
    denom = pool.tile([128, 1], F32, tag="rt_denom")
    nc.vector.reduce_sum(out=denom, in_=top8[:, 0:4], axis=AX)
    w = pool.tile([128, 1], F32, tag="rt_w")
    nc.vector.tensor_scalar_add(denom, denom, 1e-20)
    nc.vector.reciprocal(w, denom)
    nc.vector.tensor_scalar_mul(w, w, SCALE)

    # sel_w = (masked >= t4) * w ; comb = sel_w * masked
    selw = pool.tile([128, 16], F32, tag="rt_selw")
    nc.vector.tensor_scalar(selw, masked, top8[:, 3:4], w, op0=OP.is_ge, op1=OP.mult)
    nc.vector.tensor_mul(comb, selw, masked)


def build_program():
    nc = bacc.Bacc(
        "TRN2",
        target_bir_lowering=False,
        debug=False,
        enable_asserts=False,
        num_devices=NCORES,
    )

    xT = nc.dram_tensor("xT", [H, N], F32R, kind="ExternalInput").ap()
    xTf = nc.dram_tensor("xTf", [H, N], F32, kind="ExternalInput").ap()
    gk = nc.dram_tensor("gk", [H, E], F32, kind="ExternalInput").ap()
    gbr = nc.dram_tensor("gbr", [128, E], F32, kind="ExternalInput").ap()
    wg = nc.dram_tensor("wg", [EPC, H, M], F32R, kind="ExternalInput").ap()
    wu = nc.dram_tensor("wu", [EPC, H, M], F32R, kind="ExternalInput").ap()
    wd = nc.dram_tensor("wd", [EPC, M, H], F32R, kind="ExternalInput").ap()
    sg = nc.dram_tensor("sg", [H, SM], F32R, kind="ExternalInput").ap()
    su = nc.dram_tensor("su", [H, SM], F32R, kind="ExternalInput").ap()
    sd = nc.dram_tensor("sd", [SM, H], F32R, kind="ExternalInput").ap()
    sel_in = nc.dram_tensor("sel", [EPC, E, 128], F32R, kind="ExternalInput").ap()
    out = nc.dram_tensor("out", [128, N], F32, kind="ExternalOutput").ap()

    with tile.TileContext(nc) as tc:
        with (
            tc.tile_pool(name="w", bufs=1) as wpool,
            tc.tile_pool(name="sb", bufs=2) as sb,
            tc.tile_pool(name="rt", bufs=2) as rt,
            tc.tile_pool(name="ps", bufs=2, space="PSUM") as ps,
            tc.tile_pool(name="dram", bufs=1, space="DRAM") as dram,
        ):
            # ---- gating-critical small DMAs first ----
            gk_sb = wpool.tile([128, KT * E], F32, tag="gk")
            for k in range(KT):
                nc.sync.dma_start(
                    out=gk_sb[:, k * E:(k + 1) * E],
                    in_=gk[k * 128:(k + 1) * 128, :],
                )
            gbr_sb = wpool.tile([128, E], F32, tag="gbr")
            nc.sync.dma_start(out=gbr_sb, in_=gbr)
            selm_sb = wpool.tile([E, EPC * 128], F32R, tag="selm")
            for e in range(EPC):
                nc.sync.dma_start(
                    out=selm_sb[:, e * 128:(e + 1) * 128], in_=sel_in[e]
                )
            ident = wpool.tile([128, 128], F32, tag="ident")
            make_identity(nc, ident)

            def emit_xt_dma(c):
                W = CHW[c]
                off = CHOFF[c]
                xt_t = sb.tile([128, KT * W], F32R, tag="xt",
                               padded_shape=[128, KT * 512])
                xtf_t = sb.tile([128, KT * W], F32, tag="xtf", bufs=1,
                                padded_shape=[128, KT * 512])
                for k in range(KT):
                    nc.sync.dma_start(
                        out=xtf_t[:, k * W:(k + 1) * W],
                        in_=xTf[k * 128:(k + 1) * 128, off:off + W],
                    )
                for k in range(KT):
                    nc.sync.dma_start(
                        out=xt_t[:, k * W:(k + 1) * W],
                        in_=xT[k * 128:(k + 1) * 128, off:off + W],
                    )
                return xt_t, xtf_t

            xts = {0: emit_xt_dma(0)}

            # ---- resident weights (after chunk-0 activations) ----
            wg_sb = []
            wu_sb = []
            wd_sb = []
            for e in range(EPC):
                g_t = wpool.tile([128, KT * M], F32R, name=f"wg_sb{e}", tag=f"wg{e}")
                u_t = wpool.tile([128, KT * M], F32R, name=f"wu_sb{e}", tag=f"wu{e}")
                for k in range(KT):
                    nc.sync.dma_start(
                        out=g_t[:, k * M:(k + 1) * M],
                        in_=wg[e, k * 128:(k + 1) * 128, :],
                    )
                    nc.sync.dma_start(
                        out=u_t[:, k * M:(k + 1) * M],
                        in_=wu[e, k * 128:(k + 1) * 128, :],
                    )
                wg_sb.append(g_t)
                wu_sb.append(u_t)

            sg_sb = wpool.tile([128, KT * SM], F32R, tag="sg")
            su_sb = wpool.tile([128, KT * SM], F32R, tag="su")
            for k in range(KT):
                nc.sync.dma_start(
                    out=sg_sb[:, k * SM:(k + 1) * SM],
                    in_=sg[k * 128:(k + 1) * 128, :],
                )
                nc.sync.dma_start(
                    out=su_sb[:, k * SM:(k + 1) * SM],
                    in_=su[k * 128:(k + 1) * 128, :],
                )

            for e in range(EPC):
                d_t = wpool.tile([128, MB * H], F32R, name=f"wd_sb{e}", tag=f"wd{e}")
                for mb in range(MB):
                    nc.sync.dma_start(
                        out=d_t[:, mb * H:(mb + 1) * H],
                        in_=wd[e, mb * 128:(mb + 1) * 128, :],
                    )
                wd_sb.append(d_t)
            sd_sb = wpool.tile([128, H], F32R, tag="sd")
            nc.sync.dma_start(out=sd_sb, in_=sd)

            def emit_gating(c, xtf_t):
                """fp32 token-major logits + routing; returns comb tiles."""
                W = CHW[c]
                combs = []
                for t in range(W // 128):
                    plt = ps.tile([128, E], F32, tag="pmisc")
                    for k in range(KT):
                        nc.tensor.matmul(
                            plt,
                            lhsT=xtf_t[:, k * W + t * 128: k * W + (t + 1) * 128],
                            rhs=gk_sb[:, k * E:(k + 1) * E],
                            start=(k == 0),
                            stop=(k == KT - 1),
                        )
                    s4c = rt.tile([128, E], F32, tag="rt_s4c")
                    nc.scalar.activation(s4c, plt, ACT.Sigmoid)
                    nc.vector.tensor_add(s4c, s4c, gbr_sb)
                    comb = rt.tile([128, E], F32, tag="rt_comb", bufs=8)
                    _routing(nc, rt, s4c, comb)
                    combs.append(comb)
                return combs

            def emit_gating_pe_tail(c, combs):
                """transpose combine + broadcast local experts' rows."""
                W = CHW[c]
                combT = sb.tile([E, W], F32R, tag="combT",
                                padded_shape=[E, 512])
                for t, comb in enumerate(combs):
                    pct = ps.tile([E, 128], F32, tag="pmisc")
                    nc.tensor.transpose(pct, comb, ident)
                    nc.scalar.copy(combT[:, t * 128:(t + 1) * 128], pct)
                cbc = []
                for e in range(EPC):
                    pb = ps.tile([128, W], F32, tag="pmisc",
                                 padded_shape=[128, 512])
                    nc.tensor.matmul(
                        pb,
                        lhsT=selm_sb[:, e * 128:(e + 1) * 128],
                        rhs=combT,
                        start=True,
                        stop=True,
                    )
                    cb = sb.tile([128, W], F32, tag=f"cbc{e}",
                                 padded_shape=[128, 512])
                    nc.scalar.copy(cb, pb)
                    cbc.append(cb)
                return cbc

            def emit_expert_gu(c, e, xt_t, cbc_e):
                """g/u projections + inter = silu(g) * u * combine."""
                W = CHW[c]
                it = sb.tile([128, MB * W], F32R, tag=f"inter{e}", bufs=1,
                             padded_shape=[128, MB * 512])
                for mb in range(MB):
                    pg = ps.tile([128, W], F32, tag="pg", padded_shape=[128, 512])
                    for k in range(KT):
                        nc.tensor.matmul(
                            pg,
                            lhsT=wg_sb[e][:, k * M + mb * 128: k * M + (mb + 1) * 128],
                            rhs=xt_t[:, k * W:(k + 1) * W],
                            start=(k == 0),
                            stop=(k == KT - 1),
                        )
                    pu = ps.tile([128, W], F32, tag="pu", padded_shape=[128, 512])
                    for k in range(KT):
                        nc.tensor.matmul(
                            pu,
                            lhsT=wu_sb[e][:, k * M + mb * 128: k * M + (mb + 1) * 128],
                            rhs=xt_t[:, k * W:(k + 1) * W],
                            start=(k == 0),
                            stop=(k == KT - 1),
                        )
                    sig_t = sb.tile([128, W], F32, tag="sig",
                                    padded_shape=[128, 512])
                    nc.scalar.activation(sig_t, pg, ACT.Sigmoid)
                    sg_t = sb.tile([128, W], F32, tag="silu",
                                   padded_shape=[128, 512])
                    nc.vector.scalar_tensor_tensor(
                        sg_t, pg, 1.0, sig_t, op0=OP.mult, op1=OP.mult
                    )
                    us = sb.tile([128, W], F32, tag="us", padded_shape=[128, 512])
                    nc.vector.tensor_mul(us, pu, cbc_e)
                    nc.vector.tensor_mul(it[:, mb * W:(mb + 1) * W], sg_t, us)
                return it

            def emit_shared(c, xt_t):
                W = CHW[c]
                pgs = ps.tile([128, W], F32, tag="pg", padded_shape=[128, 512])
                for k in range(KT):
                    nc.tensor.matmul(
                        pgs,
                        lhsT=sg_sb[:, k * SM:(k + 1) * SM],
                        rhs=xt_t[:, k * W:(k + 1) * W],
                        start=(k == 0),
                        stop=(k == KT - 1),
                    )
                pus = ps.tile([128, W], F32, tag="pu", padded_shape=[128, 512])
                for k in range(KT):
                    nc.tensor.matmul(
                        pus,
                        lhsT=su_sb[:, k * SM:(k + 1) * SM],
                        rhs=xt_t[:, k * W:(k + 1) * W],
                        start=(k == 0),
                        stop=(k == KT - 1),
                    )
                sig_s = sb.tile([128, W], F32, tag="sig", padded_shape=[128, 512])
                nc.scalar.activation(sig_s, pgs, ACT.Sigmoid)
                sgs = sb.tile([128, W], F32, tag="silu", padded_shape=[128, 512])
                nc.vector.scalar_tensor_tensor(
                    sgs, pgs, 1.0, sig_s, op0=OP.mult, op1=OP.mult
                )
                inter_s = sb.tile([128, W], F32R, tag="inter_s",
                                  padded_shape=[128, 512])
                nc.vector.tensor_mul(inter_s, sgs, pus)
                return inter_s

            def emit_down_and_rs(c, inters, inter_s):
                W = CHW[c]
                off = CHOFF[c]
                ypart = dram.tile([H, W], F32, name=f"ypart{c}")
                for ht in range(HT):
                    py = ps.tile([128, W], F32, tag="py", padded_shape=[128, 512])
                    first = True
                    for e in range(EPC):
                        for mb in range(MB):
                            nc.tensor.matmul(
                                py,
                                lhsT=wd_sb[e][:, mb * H + ht * 128: mb * H + (ht + 1) * 128],
                                rhs=inters[e][:, mb * W:(mb + 1) * W],
                                start=first,
                                stop=False,
                            )
                            first = False
                    nc.tensor.matmul(
                        py,
                        lhsT=sd_sb[:, ht * 128:(ht + 1) * 128],
                        rhs=inter_s,
                        start=False,
                        stop=True,
                    )
                    yp = sb.tile([128, W], F32, tag="yp", padded_shape=[128, 512])
                    nc.vector.tensor_copy(yp, py)
                    nc.sync.dma_start(
                        out=ypart[ht * 128:(ht + 1) * 128, :], in_=yp
                    )
                rs_out = dram.tile([128, W], F32, name=f"rsout{c}")
                nc.gpsimd.collective_compute(
                    "ReduceScatter",
                    OP.add,
                    replica_groups=[list(range(NCORES))],
                    ins=[ypart.opt()],
                    outs=[rs_out.opt()],
                )
                nc.sync.dma_start(out=out[:, off:off + W], in_=rs_out)

            # ---- software-pipelined main loop (gating one chunk ahead) ----
            cbc = {0: emit_gating_pe_tail(0, emit_gating(0, xts[0][1]))}
            xts[1] = emit_xt_dma(1)

            for c in range(NCH):
                xt_t = xts[c][0]
                i0 = emit_expert_gu(c, 0, xt_t, cbc[c][0])
                combs_next = None
                if c + 1 < NCH:
                    combs_next = emit_gating(c + 1, xts[c + 1][1])
                i1 = emit_expert_gu(c, 1, xt_t, cbc[c][1])
                inter_s = emit_shared(c, xt_t)
                if c + 1 < NCH:
                    cbc[c + 1] = emit_gating_pe_tail(c + 1, combs_next)
                if c + 2 < NCH:
                    xts[c + 2] = emit_xt_dma(c + 2)
                emit_down_and_rs(c, [i0, i1], inter_s)

    nc.compile()
    return nc


_NC_CACHE = None


def _get_program():
    global _NC_CACHE
    if _NC_CACHE is None:
        _NC_CACHE = build_program()
    return _NC_CACHE


def _make_in_maps(inputs):
    x = np.ascontiguousarray(
        np.asarray(inputs["hidden_states"], dtype=np.float32).reshape(N, H).T
    )
    gk = np.ascontiguousarray(np.asarray(inputs["gate_kernel"], dtype=np.float32))
    gb = np.asarray(inputs["gate_bias"], dtype=np.float32)
    gbr = np.ascontiguousarray(np.broadcast_to(gb[None, :], (128, E)))
    w_gate = np.asarray(inputs["w_gate"], dtype=np.float32)
    w_up = np.asarray(inputs["w_up"], dtype=np.float32)
    w_down = np.asarray(inputs["w_down"], dtype=np.float32)
    sw_gate = np.asarray(inputs["sw_gate"], dtype=np.float32)
    sw_up = np.asarray(inputs["sw_up"], dtype=np.float32)
    sw_down = np.asarray(inputs["sw_down"], dtype=np.float32)

    in_maps = []
    for c in range(NCORES):
        sel = np.zeros((EPC, E, 128), dtype=np.float32)
        for e in range(EPC):
            sel[e, EPC * c + e, :] = 1.0
        in_maps.append({
            "xT": x,
            "xTf": x,
            "gk": gk,
            "gbr": gbr,
            "wg": np.ascontiguousarray(w_gate[EPC * c:EPC * (c + 1)]),
            "wu": np.ascontiguousarray(w_up[EPC * c:EPC * (c + 1)]),
            "wd": np.ascontiguousarray(w_down[EPC * c:EPC * (c + 1)]),
            "sg": np.ascontiguousarray(sw_gate[:, SM * c:SM * (c + 1)]),
            "su": np.ascontiguousarray(sw_up[:, SM * c:SM * (c + 1)]),
            "sd": np.ascontiguousarray(sw_down[SM * c:SM * (c + 1), :]),
            "sel": sel,
        })
    return in_maps


def run(inputs, trace=False):
    """Returns (output, BassKernelResults)."""
    nc = _get_program()
    in_maps = _make_in_maps(inputs)
    res = run_bass_kernel_spmd(
        nc, in_maps, core_ids=list(range(NCORES)), trace=trace
    )
    yT = np.concatenate(
        [res.results[c]["out"] for c in range(NCORES)], axis=0
    )
    y = np.ascontiguousarray(yT.T).reshape(2, 1024, H).astype(np.float32)
    return y, res


def kernel(**inputs):
    y, _ = run(inputs, trace=False)
    return y


# revision 8
# speedup vs baseline: 1.2806x; 1.1116x over previous
"""DeepseekV3 MoE kernel for 8 Trainium2 NeuronCores.

Sharding: expert-parallel (2 routed experts per core) + intermediate-sharded
shared expert (128 of 1024 columns per core); gate replicated (computed in
full fp32 on every core, pipelined one chunk ahead of the expert compute);
per-chunk ReduceScatter combines partial outputs; host concatenates shards.

Self-contained: hardcodes all shapes. Only dependency is the concourse
tree (on PYTHONPATH in the container) and numpy.
"""

import os
import sys

import numpy as np

for _p in ("/opt/trn_rl_repo", "/root/.axon_site/_ro/trn_rl_repo"):
    if os.path.isdir(_p) and _p not in sys.path:
        sys.path.append(_p)

import concourse.bacc as bacc
import concourse.mybir as mybir
import concourse.tile as tile
from concourse.bass_utils import run_bass_kernel_spmd
from concourse.masks import make_identity

F32 = mybir.dt.float32
F32R = mybir.dt.float32r
BF16 = mybir.dt.bfloat16
AX = mybir.AxisListType.X
OP = mybir.AluOpType
ACT = mybir.ActivationFunctionType

H = 1024          # hidden size
M = 512           # expert intermediate
E = 16            # routed experts
EPC = 2           # experts per core
NCORES = 8
N = 2048          # tokens (B*S)
KT = H // 128     # 8 contraction tiles
MB = M // 128     # 4 m-tiles per expert
HT = H // 128     # 8 output h-tiles
SCALE = 2.5
SM = 128          # shared-expert intermediate columns per core

# token chunks: smaller final chunks shrink the un-overlapped tail
# (last ReduceScatter + output DMA)
CHW = [512, 512, 512, 256, 256]
CHOFF = [0, 512, 1024, 1536, 1792]
NCH = len(CHW)


def _routing(nc, pool, s4c, comb):
    """Token-major DeepseekV3 noaux_tc routing for one [128, 16] tile.

    s4c: sigmoid(logits) + bias, [128, 16] fp32 SBUF.
    comb: output combine weights [128, 16] (SCALE * topk_weight scattered).
    """
    v = s4c.rearrange("p (g s) -> p g s", g=4)

    # sum of top-2 per group of 4 = max over the 6 pairwise sums
    pairs = pool.tile([128, 24], F32, tag="rt_pairs")
    pv = pairs.rearrange("p (g s) -> p g s", g=4)
    nc.vector.tensor_add(pv[:, :, 0:3], v[:, :, 0:3], v[:, :, 1:4])
    nc.vector.tensor_add(pv[:, :, 3:5], v[:, :, 0:2], v[:, :, 2:4])
    nc.vector.tensor_add(pv[:, :, 5:6], v[:, :, 0:1], v[:, :, 3:4])
    gsum = pool.tile([128, 4], F32, tag="rt_gsum")
    nc.vector.reduce_max(out=gsum, in_=pv, axis=AX)

    # 2nd largest group sum = max over the 6 pairwise mins
    gmins = pool.tile([128, 8], F32, tag="rt_gmins")
    nc.vector.tensor_tensor(gmins[:, 0:3], gsum[:, 0:3], gsum[:, 1:4], op=OP.min)
    nc.vector.tensor_tensor(gmins[:, 3:5], gsum[:, 0:2], gsum[:, 2:4], op=OP.min)
    nc.vector.tensor_tensor(gmins[:, 5:6], gsum[:, 0:1], gsum[:, 3:4], op=OP.min)
    t2g = pool.tile([128, 1], F32, tag="rt_t2g")
    nc.vector.reduce_max(out=t2g, in_=gmins[:, 0:6], axis=AX)

    # group mask (1.0 for the top-2 groups), expanded to 16 experts
    gmask = pool.tile([128, 4], F32, tag="rt_gmask")
    nc.vector.tensor_scalar(gmask, gsum, t2g, None, op0=OP.is_ge)
    mask16 = pool.tile([128, 16], F32, tag="rt_mask16")
    m16v = mask16.rearrange("p (g s) -> p g s", g=4)
    for j in range(4):
        nc.vector.tensor_copy(m16v[:, :, j], gmask)

    masked = pool.tile([128, 16], F32, tag="rt_masked")
    nc.vector.tensor_mul(masked, s4c, mask16)

    # top-4 of 16 via Max8, threshold select, normalize
    top8 = pool.tile([128, 8], F32, tag="rt_top8")
    nc.vector.max(out=top8, in_=masked)
    denom = pool.tile([128, 1], F32, tag="rt_denom")
    nc.vector.reduce_sum(out=denom, in_=top8[:, 0:4], axis=AX)
    w = pool.tile([128, 1], F32, tag="rt_w")
    nc.vector.tensor_scalar_add(denom, denom, 1e-20)
    nc.vector.reciprocal(w, denom)
    nc.vector.tensor_scalar_mul(w, w, SCALE)

    # sel_w = (masked >= t4) * w ; comb = sel_w * masked
    selw = pool.tile([128, 16], F32, tag="rt_selw")
    nc.vector.tensor_scalar(selw, masked, top8[:, 3:4], w, op0=OP.is_ge, op1=OP.mult)
    nc.vector.tensor_mul(comb, selw, masked)


def build_program():
    nc = bacc.Bacc(
        "TRN2",
        target_bir_lowering=False,
        debug=False,
        enable_asserts=False,
        num_devices=NCORES,
    )

    xT = nc.dram_tensor("xT", [H, N], BF16, kind="ExternalInput").ap()
    xTf = nc.dram_tensor("xTf", [H, N], F32, kind="ExternalInput").ap()
    gk = nc.dram_tensor("gk", [H, E], F32, kind="ExternalInput").ap()
    gbr = nc.dram_tensor("gbr", [128, E], F32, kind="ExternalInput").ap()
    wg = nc.dram_tensor("wg", [EPC, H, M], BF16, kind="ExternalInput").ap()
    wu = nc.dram_tensor("wu", [EPC, H, M], BF16, kind="ExternalInput").ap()
    wd = nc.dram_tensor("wd", [EPC, M, H], BF16, kind="ExternalInput").ap()
    sg = nc.dram_tensor("sg", [H, SM], BF16, kind="ExternalInput").ap()
    su = nc.dram_tensor("su", [H, SM], BF16, kind="ExternalInput").ap()
    sd = nc.dram_tensor("sd", [SM, H], BF16, kind="ExternalInput").ap()
    sel_in = nc.dram_tensor("sel", [EPC, E, 128], F32R, kind="ExternalInput").ap()
    out = nc.dram_tensor("out", [128, N], F32, kind="ExternalOutput").ap()

    with tile.TileContext(nc) as tc:
        with (
            tc.tile_pool(name="w", bufs=1) as wpool,
            tc.tile_pool(name="sb", bufs=2) as sb,
            tc.tile_pool(name="rt", bufs=2) as rt,
            tc.tile_pool(name="ps", bufs=2, space="PSUM") as ps,
            tc.tile_pool(name="dram", bufs=1, space="DRAM") as dram,
        ):
            # ---- gating-critical small DMAs first ----
            gk_sb = wpool.tile([128, KT * E], F32, tag="gk")
            for k in range(KT):
                nc.sync.dma_start(
                    out=gk_sb[:, k * E:(k + 1) * E],
                    in_=gk[k * 128:(k + 1) * 128, :],
                )
            gbr_sb = wpool.tile([128, E], F32, tag="gbr")
            nc.sync.dma_start(out=gbr_sb, in_=gbr)
            selm_sb = wpool.tile([E, EPC * 128], F32R, tag="selm")
            for e in range(EPC):
                nc.sync.dma_start(
                    out=selm_sb[:, e * 128:(e + 1) * 128], in_=sel_in[e]
                )
            ident = wpool.tile([128, 128], F32, tag="ident")
            make_identity(nc, ident)

            def emit_xt_dma(c):
                W = CHW[c]
                off = CHOFF[c]
                xt_t = sb.tile([128, KT * W], BF16, tag="xt",
                               padded_shape=[128, KT * 512])
                xtf_t = sb.tile([128, KT * W], F32, tag="xtf", bufs=1,
                                padded_shape=[128, KT * 512])
                for k in range(KT):
                    nc.sync.dma_start(
                        out=xtf_t[:, k * W:(k + 1) * W],
                        in_=xTf[k * 128:(k + 1) * 128, off:off + W],
                    )
                for k in range(KT):
                    nc.sync.dma_start(
                        out=xt_t[:, k * W:(k + 1) * W],
                        in_=xT[k * 128:(k + 1) * 128, off:off + W],
                    )
                return xt_t, xtf_t

            xts = {0: emit_xt_dma(0)}

            # ---- resident weights (after chunk-0 activations) ----
            wg_sb = []
            wu_sb = []
            wd_sb = []
            for e in range(EPC):
                g_t = wpool.tile([128, KT * M], BF16, name=f"wg_sb{e}", tag=f"wg{e}")
                u_t = wpool.tile([128, KT * M], BF16, name=f"wu_sb{e}", tag=f"wu{e}")
                for k in range(KT):
                    nc.sync.dma_start(
                        out=g_t[:, k * M:(k + 1) * M],
                        in_=wg[e, k * 128:(k + 1) * 128, :],
                    )
                    nc.sync.dma_start(
                        out=u_t[:, k * M:(k + 1) * M],
                        in_=wu[e, k * 128:(k + 1) * 128, :],
                    )
                wg_sb.append(g_t)
                wu_sb.append(u_t)

            sg_sb = wpool.tile([128, KT * SM], BF16, tag="sg")
            su_sb = wpool.tile([128, KT * SM], BF16, tag="su")
            for k in range(KT):
                nc.sync.dma_start(
                    out=sg_sb[:, k * SM:(k + 1) * SM],
                    in_=sg[k * 128:(k + 1) * 128, :],
                )
                nc.sync.dma_start(
                    out=su_sb[:, k * SM:(k + 1) * SM],
                    in_=su[k * 128:(k + 1) * 128, :],
                )

            for e in range(EPC):
                d_t = wpool.tile([128, MB * H], BF16, name=f"wd_sb{e}", tag=f"wd{e}")
                for mb in range(MB):
                    nc.sync.dma_start(
                        out=d_t[:, mb * H:(mb + 1) * H],
                        in_=wd[e, mb * 128:(mb + 1) * 128, :],
                    )
                wd_sb.append(d_t)
            sd_sb = wpool.tile([128, H], BF16, tag="sd")
            nc.sync.dma_start(out=sd_sb, in_=sd)

            def emit_gating(c, xtf_t):
                """fp32 token-major logits + routing; returns comb tiles."""
                W = CHW[c]
                combs = []
                for t in range(W // 128):
                    plt = ps.tile([128, E], F32, tag="pmisc")
                    for k in range(KT):
                        nc.tensor.matmul(
                            plt,
                            lhsT=xtf_t[:, k * W + t * 128: k * W + (t + 1) * 128],
                            rhs=gk_sb[:, k * E:(k + 1) * E],
                            start=(k == 0),
                            stop=(k == KT - 1),
                        )
                    s4c = rt.tile([128, E], F32, tag="rt_s4c")
                    nc.scalar.activation(s4c, plt, ACT.Sigmoid)
                    nc.vector.tensor_add(s4c, s4c, gbr_sb)
                    comb = rt.tile([128, E], F32, tag="rt_comb", bufs=8)
                    _routing(nc, rt, s4c, comb)
                    combs.append(comb)
                return combs

            def emit_gating_pe_tail(c, combs):
                """transpose combine + broadcast local experts' rows."""
                W = CHW[c]
                combT = sb.tile([E, W], F32R, tag="combT",
                                padded_shape=[E, 512])
                for t, comb in enumerate(combs):
                    pct = ps.tile([E, 128], F32, tag="pmisc")
                    nc.tensor.transpose(pct, comb, ident)
                    nc.scalar.copy(combT[:, t * 128:(t + 1) * 128], pct)
                cbc = []
                for e in range(EPC):
                    pb = ps.tile([128, W], F32, tag="pmisc",
                                 padded_shape=[128, 512])
                    nc.tensor.matmul(
                        pb,
                        lhsT=selm_sb[:, e * 128:(e + 1) * 128],
                        rhs=combT,
                        start=True,
                        stop=True,
                    )
                    cb = sb.tile([128, W], F32, tag=f"cbc{e}",
                                 padded_shape=[128, 512])
                    nc.scalar.copy(cb, pb)
                    cbc.append(cb)
                return cbc

            def emit_expert_gu(c, e, xt_t, cbc_e):
                """g/u projections + inter = silu(g) * u * combine."""
                W = CHW[c]
                it = sb.tile([128, MB * W], BF16, tag=f"inter{e}", bufs=1,
                             padded_shape=[128, MB * 512])
                for mb in range(MB):
                    pg = ps.tile([128, W], F32, tag="pg", padded_shape=[128, 512])
                    for k in range(KT):
                        nc.tensor.matmul(
                            pg,
                            lhsT=wg_sb[e][:, k * M + mb * 128: k * M + (mb + 1) * 128],
                            rhs=xt_t[:, k * W:(k + 1) * W],
                            start=(k == 0),
                            stop=(k == KT - 1),
                        )
                    pu = ps.tile([128, W], F32, tag="pu", padded_shape=[128, 512])
                    for k in range(KT):
                        nc.tensor.matmul(
                            pu,
                            lhsT=wu_sb[e][:, k * M + mb * 128: k * M + (mb + 1) * 128],
                            rhs=xt_t[:, k * W:(k + 1) * W],
                            start=(k == 0),
                            stop=(k == KT - 1),
                        )
                    sig_t = sb.tile([128, W], BF16, tag="sig",
                                    padded_shape=[128, 512])
                    nc.scalar.activation(sig_t, pg, ACT.Sigmoid)
                    sg_t = sb.tile([128, W], BF16, tag="silu",
                                   padded_shape=[128, 512])
                    nc.vector.scalar_tensor_tensor(
                        sg_t, pg, 1.0, sig_t, op0=OP.mult, op1=OP.mult
                    )
                    us = sb.tile([128, W], BF16, tag="us", padded_shape=[128, 512])
                    nc.vector.tensor_mul(us, pu, cbc_e)
                    nc.vector.tensor_mul(it[:, mb * W:(mb + 1) * W], sg_t, us)
                return it

            def emit_shared(c, xt_t):
                W = CHW[c]
                pgs = ps.tile([128, W], F32, tag="pg", padded_shape=[128, 512])
                for k in range(KT):
                    nc.tensor.matmul(
                        pgs,
                        lhsT=sg_sb[:, k * SM:(k + 1) * SM],
                        rhs=xt_t[:, k * W:(k + 1) * W],
                        start=(k == 0),
                        stop=(k == KT - 1),
                    )
                pus = ps.tile([128, W], F32, tag="pu", padded_shape=[128, 512])
                for k in range(KT):
                    nc.tensor.matmul(
                        pus,
                        lhsT=su_sb[:, k * SM:(k + 1) * SM],
                        rhs=xt_t[:, k * W:(k + 1) * W],
                        start=(k == 0),
                        stop=(k == KT - 1),
                    )
                sig_s = sb.tile([128, W], BF16, tag="sig", padded_shape=[128, 512])
                nc.scalar.activation(sig_s, pgs, ACT.Sigmoid)
                sgs = sb.tile([128, W], BF16, tag="silu", padded_shape=[128, 512])
                nc.vector.scalar_tensor_tensor(
                    sgs, pgs, 1.0, sig_s, op0=OP.mult, op1=OP.mult
                )
                inter_s = sb.tile([128, W], BF16, tag="inter_s",
                                  padded_shape=[128, 512])
                nc.vector.tensor_mul(inter_s, sgs, pus)
                return inter_s

            def emit_down_and_rs(c, inters, inter_s):
                W = CHW[c]
                off = CHOFF[c]
                ypart = dram.tile([H, W], F32, name=f"ypart{c}", tag=f"ypart{c}")
                for ht in range(HT):
                    py = ps.tile([128, W], F32, tag="py", padded_shape=[128, 512])
                    first = True
                    for e in range(EPC):
                        for mb in range(MB):
                            nc.tensor.matmul(
                                py,
                                lhsT=wd_sb[e][:, mb * H + ht * 128: mb * H + (ht + 1) * 128],
                                rhs=inters[e][:, mb * W:(mb + 1) * W],
                                start=first,
                                stop=False,
                            )
                            first = False
                    nc.tensor.matmul(
                        py,
                        lhsT=sd_sb[:, ht * 128:(ht + 1) * 128],
                        rhs=inter_s,
                        start=False,
                        stop=True,
                    )
                    yp = sb.tile([128, W], F32, tag="yp", padded_shape=[128, 512])
                    nc.vector.tensor_copy(yp, py)
                    nc.sync.dma_start(
                        out=ypart[ht * 128:(ht + 1) * 128, :], in_=yp
                    )
                rs_out = dram.tile([128, W], F32, name=f"rsout{c}", tag=f"rsout{c}")
                nc.gpsimd.collective_compute(
                    "ReduceScatter",
                    OP.add,
                    replica_groups=[list(range(NCORES))],
                    ins=[ypart.opt()],
                    outs=[rs_out.opt()],
                )
                nc.sync.dma_start(out=out[:, off:off + W], in_=rs_out)

            # ---- software-pipelined main loop (gating one chunk ahead) ----
            cbc = {0: emit_gating_pe_tail(0, emit_gating(0, xts[0][1]))}
            xts[1] = emit_xt_dma(1)

            for c in range(NCH):
                xt_t = xts[c][0]
                i0 = emit_expert_gu(c, 0, xt_t, cbc[c][0])
                combs_next = None
                if c + 1 < NCH:
                    combs_next = emit_gating(c + 1, xts[c + 1][1])
                i1 = emit_expert_gu(c, 1, xt_t, cbc[c][1])
                inter_s = emit_shared(c, xt_t)
                if c + 1 < NCH:
                    cbc[c + 1] = emit_gating_pe_tail(c + 1, combs_next)
                if c + 2 < NCH:
                    xts[c + 2] = emit_xt_dma(c + 2)
                emit_down_and_rs(c, [i0, i1], inter_s)

    nc.compile()
    return nc


_NC_CACHE = None


def _get_program():
    global _NC_CACHE
    if _NC_CACHE is None:
        _NC_CACHE = build_program()
    return _NC_CACHE


def _make_in_maps(inputs):
    import ml_dtypes
    bf16 = ml_dtypes.bfloat16
    x = np.ascontiguousarray(
        np.asarray(inputs["hidden_states"], dtype=np.float32).reshape(N, H).T
    )
    x_bf = x.astype(bf16)
    gk = np.ascontiguousarray(np.asarray(inputs["gate_kernel"], dtype=np.float32))
    gb = np.asarray(inputs["gate_bias"], dtype=np.float32)
    gbr = np.ascontiguousarray(np.broadcast_to(gb[None, :], (128, E)))
    w_gate = np.asarray(inputs["w_gate"], dtype=np.float32)
    w_up = np.asarray(inputs["w_up"], dtype=np.float32)
    w_down = np.asarray(inputs["w_down"], dtype=np.float32)
    sw_gate = np.asarray(inputs["sw_gate"], dtype=np.float32)
    sw_up = np.asarray(inputs["sw_up"], dtype=np.float32)
    sw_down = np.asarray(inputs["sw_down"], dtype=np.float32)

    in_maps = []
    for c in range(NCORES):
        sel = np.zeros((EPC, E, 128), dtype=np.float32)
        for e in range(EPC):
            sel[e, EPC * c + e, :] = 1.0
        in_maps.append({
            "xT": x_bf,
            "xTf": x,
            "gk": gk,
            "gbr": gbr,
            "wg": np.ascontiguousarray(w_gate[EPC * c:EPC * (c + 1)]).astype(bf16),
            "wu": np.ascontiguousarray(w_up[EPC * c:EPC * (c + 1)]).astype(bf16),
            "wd": np.ascontiguousarray(w_down[EPC * c:EPC * (c + 1)]).astype(bf16),
            "sg": np.ascontiguousarray(sw_gate[:, SM * c:SM * (c + 1)]).astype(bf16),
            "su": np.ascontiguousarray(sw_up[:, SM * c:SM * (c + 1)]).astype(bf16),
            "sd": np.ascontiguousarray(sw_down[SM * c:SM * (c + 1), :]).astype(bf16),
            "sel": sel,
        })
    return in_maps


def run(inputs, trace=False):
    """Returns (output, BassKernelResults)."""
    nc = _get_program()
    in_maps = _make_in_maps(inputs)
    res = run_bass_kernel_spmd(
        nc, in_maps, core_ids=list(range(NCORES)), trace=trace
    )
    yT = np.concatenate(
        [res.results[c]["out"] for c in range(NCORES)], axis=0
    )
    y = np.ascontiguousarray(yT.T).reshape(2, 1024, H).astype(np.float32)
    return y, res


def kernel(**inputs):
    y, _ = run(inputs, trace=False)
    return y


# revision 9
# speedup vs baseline: 1.2830x; 1.0018x over previous
"""DeepseekV3 MoE kernel for 8 Trainium2 NeuronCores.

Sharding: expert-parallel (2 routed experts per core) + intermediate-sharded
shared expert (128 of 1024 columns per core); gate replicated (computed in
full fp32 on every core, pipelined one chunk ahead of the expert compute);
per-chunk ReduceScatter combines partial outputs; host concatenates shards.

Self-contained: hardcodes all shapes. Only dependency is the concourse
tree (on PYTHONPATH in the container) and numpy.
"""

import os
import sys

import numpy as np

for _p in ("/opt/trn_rl_repo", "/root/.axon_site/_ro/trn_rl_repo"):
    if os.path.isdir(_p) and _p not in sys.path:
        sys.path.append(_p)

import concourse.bacc as bacc
import concourse.mybir as mybir
import concourse.tile as tile
from concourse.bass_utils import run_bass_kernel_spmd
from concourse.masks import make_identity

F32 = mybir.dt.float32
F32R = mybir.dt.float32r
BF16 = mybir.dt.bfloat16
AX = mybir.AxisListType.X
OP = mybir.AluOpType
ACT = mybir.ActivationFunctionType

H = 1024          # hidden size
M = 512           # expert intermediate
E = 16            # routed experts
EPC = 2           # experts per core
NCORES = 8
N = 2048          # tokens (B*S)
KT = H // 128     # 8 contraction tiles
MB = M // 128     # 4 m-tiles per expert
HT = H // 128     # 8 output h-tiles
SCALE = 2.5
SM = 128          # shared-expert intermediate columns per core

# token chunks: smaller final chunks shrink the un-overlapped tail
# (last ReduceScatter + output DMA)
CHW = [512, 512, 512, 256, 256]
CHOFF = [0, 512, 1024, 1536, 1792]
NCH = len(CHW)


def _routing(nc, pool, s4c, comb):
    """Token-major DeepseekV3 noaux_tc routing for one [128, 16] tile.

    s4c: sigmoid(logits) + bias, [128, 16] fp32 SBUF.
    comb: output combine weights [128, 16] (SCALE * topk_weight scattered).
    """
    v = s4c.rearrange("p (g s) -> p g s", g=4)

    # sum of top-2 per group of 4 = max over the 6 pairwise sums
    pairs = pool.tile([128, 24], F32, tag="rt_pairs")
    pv = pairs.rearrange("p (g s) -> p g s", g=4)
    nc.vector.tensor_add(pv[:, :, 0:3], v[:, :, 0:3], v[:, :, 1:4])
    nc.vector.tensor_add(pv[:, :, 3:5], v[:, :, 0:2], v[:, :, 2:4])
    nc.vector.tensor_add(pv[:, :, 5:6], v[:, :, 0:1], v[:, :, 3:4])
    gsum = pool.tile([128, 4], F32, tag="rt_gsum")
    nc.vector.reduce_max(out=gsum, in_=pv, axis=AX)

    # 2nd largest group sum = max over the 6 pairwise mins
    gmins = pool.tile([128, 8], F32, tag="rt_gmins")
    nc.vector.tensor_tensor(gmins[:, 0:3], gsum[:, 0:3], gsum[:, 1:4], op=OP.min)
    nc.vector.tensor_tensor(gmins[:, 3:5], gsum[:, 0:2], gsum[:, 2:4], op=OP.min)
    nc.vector.tensor_tensor(gmins[:, 5:6], gsum[:, 0:1], gsum[:, 3:4], op=OP.min)
    t2g = pool.tile([128, 1], F32, tag="rt_t2g")
    nc.vector.reduce_max(out=t2g, in_=gmins[:, 0:6], axis=AX)

    # group mask (1.0 for the top-2 groups), expanded to 16 experts
    gmask = pool.tile([128, 4], F32, tag="rt_gmask")
    nc.vector.tensor_scalar(gmask, gsum, t2g, None, op0=OP.is_ge)
    mask16 = pool.tile([128, 16], F32, tag="rt_mask16")
    m16v = mask16.rearrange("p (g s) -> p g s", g=4)
    for j in range(4):
        nc.vector.tensor_copy(m16v[:, :, j], gmask)

    masked = pool.tile([128, 16], F32, tag="rt_masked")
    nc.vector.tensor_mul(masked, s4c, mask16)

    # top-4 of 16 via Max8, threshold select, normalize
    top8 = pool.tile([128, 8], F32, tag="rt_top8")
    nc.vector.max(out=top8, in_=masked)
    denom = pool.tile([128, 1], F32, tag="rt_denom")
    nc.vector.reduce_sum(out=denom, in_=top8[:, 0:4], axis=AX)
    w = pool.tile([128, 1], F32, tag="rt_w")
    nc.vector.tensor_scalar_add(denom, denom, 1e-20)
    nc.vector.reciprocal(w, denom)
    nc.vector.tensor_scalar_mul(w, w, SCALE)

    # sel_w = (masked >= t4) * w ; comb = sel_w * masked
    selw = pool.tile([128, 16], F32, tag="rt_selw")
    nc.vector.tensor_scalar(selw, masked, top8[:, 3:4], w, op0=OP.is_ge, op1=OP.mult)
    nc.vector.tensor_mul(comb, selw, masked)


def build_program():
    nc = bacc.Bacc(
        "TRN2",
        target_bir_lowering=False,
        debug=False,
        enable_asserts=False,
        num_devices=NCORES,
    )

    xT = nc.dram_tensor("xT", [H, N], BF16, kind="ExternalInput").ap()
    xTf = nc.dram_tensor("xTf", [H, N], F32, kind="ExternalInput").ap()
    gk = nc.dram_tensor("gk", [H, E], F32, kind="ExternalInput").ap()
    gbr = nc.dram_tensor("gbr", [128, E], F32, kind="ExternalInput").ap()
    wg = nc.dram_tensor("wg", [EPC, H, M], BF16, kind="ExternalInput").ap()
    wu = nc.dram_tensor("wu", [EPC, H, M], BF16, kind="ExternalInput").ap()
    wd = nc.dram_tensor("wd", [EPC, M, H], BF16, kind="ExternalInput").ap()
    sg = nc.dram_tensor("sg", [H, SM], BF16, kind="ExternalInput").ap()
    su = nc.dram_tensor("su", [H, SM], BF16, kind="ExternalInput").ap()
    sd = nc.dram_tensor("sd", [SM, H], BF16, kind="ExternalInput").ap()
    sel_in = nc.dram_tensor("sel", [EPC, E, 128], F32R, kind="ExternalInput").ap()
    out = nc.dram_tensor("out", [128, N], F32, kind="ExternalOutput").ap()

    with tile.TileContext(nc) as tc:
        with (
            tc.tile_pool(name="w", bufs=1) as wpool,
            tc.tile_pool(name="sb", bufs=2) as sb,
            tc.tile_pool(name="rt", bufs=2) as rt,
            tc.tile_pool(name="ps", bufs=2, space="PSUM") as ps,
            tc.tile_pool(name="dram", bufs=1, space="DRAM") as dram,
        ):
            # ---- gating-critical small DMAs first ----
            gk_sb = wpool.tile([128, KT * E], F32, tag="gk")
            for k in range(KT):
                nc.sync.dma_start(
                    out=gk_sb[:, k * E:(k + 1) * E],
                    in_=gk[k * 128:(k + 1) * 128, :],
                )
            gbr_sb = wpool.tile([128, E], F32, tag="gbr")
            nc.sync.dma_start(out=gbr_sb, in_=gbr)
            selm_sb = wpool.tile([E, EPC * 128], F32R, tag="selm")
            for e in range(EPC):
                nc.sync.dma_start(
                    out=selm_sb[:, e * 128:(e + 1) * 128], in_=sel_in[e]
                )
            ident = wpool.tile([128, 128], F32, tag="ident")
            make_identity(nc, ident)

            def emit_xt_dma(c):
                W = CHW[c]
                off = CHOFF[c]
                xt_t = sb.tile([128, KT * W], BF16, tag="xt", bufs=3,
                               padded_shape=[128, KT * 512])
                xtf_t = sb.tile([128, KT * W], F32, tag="xtf", bufs=2,
                                padded_shape=[128, KT * 512])
                for k in range(KT):
                    nc.sync.dma_start(
                        out=xtf_t[:, k * W:(k + 1) * W],
                        in_=xTf[k * 128:(k + 1) * 128, off:off + W],
                    )
                for k in range(KT):
                    nc.sync.dma_start(
                        out=xt_t[:, k * W:(k + 1) * W],
                        in_=xT[k * 128:(k + 1) * 128, off:off + W],
                    )
                return xt_t, xtf_t

            xts = {0: emit_xt_dma(0)}

            # ---- resident weights (after chunk-0 activations) ----
            wg_sb = []
            wu_sb = []
            wd_sb = []
            for e in range(EPC):
                g_t = wpool.tile([128, KT * M], BF16, name=f"wg_sb{e}", tag=f"wg{e}")
                u_t = wpool.tile([128, KT * M], BF16, name=f"wu_sb{e}", tag=f"wu{e}")
                for k in range(KT):
                    nc.sync.dma_start(
                        out=g_t[:, k * M:(k + 1) * M],
                        in_=wg[e, k * 128:(k + 1) * 128, :],
                    )
                    nc.sync.dma_start(
                        out=u_t[:, k * M:(k + 1) * M],
                        in_=wu[e, k * 128:(k + 1) * 128, :],
                    )
                wg_sb.append(g_t)
                wu_sb.append(u_t)

            sg_sb = wpool.tile([128, KT * SM], BF16, tag="sg")
            su_sb = wpool.tile([128, KT * SM], BF16, tag="su")
            for k in range(KT):
                nc.sync.dma_start(
                    out=sg_sb[:, k * SM:(k + 1) * SM],
                    in_=sg[k * 128:(k + 1) * 128, :],
                )
                nc.sync.dma_start(
                    out=su_sb[:, k * SM:(k + 1) * SM],
                    in_=su[k * 128:(k + 1) * 128, :],
                )

            for e in range(EPC):
                d_t = wpool.tile([128, MB * H], BF16, name=f"wd_sb{e}", tag=f"wd{e}")
                for mb in range(MB):
                    nc.sync.dma_start(
                        out=d_t[:, mb * H:(mb + 1) * H],
                        in_=wd[e, mb * 128:(mb + 1) * 128, :],
                    )
                wd_sb.append(d_t)
            sd_sb = wpool.tile([128, H], BF16, tag="sd")
            nc.sync.dma_start(out=sd_sb, in_=sd)

            def emit_gating(c, xtf_t):
                """fp32 token-major logits + routing; returns comb tiles."""
                W = CHW[c]
                combs = []
                for t in range(W // 128):
                    plt = ps.tile([128, E], F32, tag="pmisc")
                    for k in range(KT):
                        nc.tensor.matmul(
                            plt,
                            lhsT=xtf_t[:, k * W + t * 128: k * W + (t + 1) * 128],
                            rhs=gk_sb[:, k * E:(k + 1) * E],
                            start=(k == 0),
                            stop=(k == KT - 1),
                        )
                    s4c = rt.tile([128, E], F32, tag="rt_s4c")
                    nc.scalar.activation(s4c, plt, ACT.Sigmoid)
                    nc.vector.tensor_add(s4c, s4c, gbr_sb)
                    comb = rt.tile([128, E], F32, tag="rt_comb", bufs=8)
                    _routing(nc, rt, s4c, comb)
                    combs.append(comb)
                return combs

            def emit_gating_pe_tail(c, combs):
                """transpose combine + broadcast local experts' rows."""
                W = CHW[c]
                combT = sb.tile([E, W], F32R, tag="combT",
                                padded_shape=[E, 512])
                for t, comb in enumerate(combs):
                    pct = ps.tile([E, 128], F32, tag="pmisc")
                    nc.tensor.transpose(pct, comb, ident)
                    nc.scalar.copy(combT[:, t * 128:(t + 1) * 128], pct)
                cbc = []
                for e in range(EPC):
                    pb = ps.tile([128, W], F32, tag="pmisc",
                                 padded_shape=[128, 512])
                    nc.tensor.matmul(
                        pb,
                        lhsT=selm_sb[:, e * 128:(e + 1) * 128],
                        rhs=combT,
                        start=True,
                        stop=True,
                    )
                    cb = sb.tile([128, W], F32, tag=f"cbc{e}",
                                 padded_shape=[128, 512])
                    nc.scalar.copy(cb, pb)
                    cbc.append(cb)
                return cbc

            def emit_expert_gu(c, e, xt_t, cbc_e):
                """g/u projections + inter = silu(g) * u * combine."""
                W = CHW[c]
                it = sb.tile([128, MB * W], BF16, tag=f"inter{e}", bufs=1,
                             padded_shape=[128, MB * 512])
                for mb in range(MB):
                    pg = ps.tile([128, W], F32, tag="pg", padded_shape=[128, 512])
                    for k in range(KT):
                        nc.tensor.matmul(
                            pg,
                            lhsT=wg_sb[e][:, k * M + mb * 128: k * M + (mb + 1) * 128],
                            rhs=xt_t[:, k * W:(k + 1) * W],
                            start=(k == 0),
                            stop=(k == KT - 1),
                        )
                    pu = ps.tile([128, W], F32, tag="pu", padded_shape=[128, 512])
                    for k in range(KT):
                        nc.tensor.matmul(
                            pu,
                            lhsT=wu_sb[e][:, k * M + mb * 128: k * M + (mb + 1) * 128],
                            rhs=xt_t[:, k * W:(k + 1) * W],
                            start=(k == 0),
                            stop=(k == KT - 1),
                        )
                    sig_t = sb.tile([128, W], BF16, tag="sig",
                                    padded_shape=[128, 512])
                    nc.scalar.activation(sig_t, pg, ACT.Sigmoid)
                    sg_t = sb.tile([128, W], BF16, tag="silu",
                                   padded_shape=[128, 512])
                    nc.vector.scalar_tensor_tensor(
                        sg_t, pg, 1.0, sig_t, op0=OP.mult, op1=OP.mult
                    )
                    us = sb.tile([128, W], BF16, tag="us", padded_shape=[128, 512])
                    nc.vector.tensor_mul(us, pu, cbc_e)
                    nc.vector.tensor_mul(it[:, mb * W:(mb + 1) * W], sg_t, us)
                return it

            def emit_shared(c, xt_t):
                W = CHW[c]
                pgs = ps.tile([128, W], F32, tag="pg", padded_shape=[128, 512])
                for k in range(KT):
                    nc.tensor.matmul(
                        pgs,
                        lhsT=sg_sb[:, k * SM:(k + 1) * SM],
                        rhs=xt_t[:, k * W:(k + 1) * W],
                        start=(k == 0),
                        stop=(k == KT - 1),
                    )
                pus = ps.tile([128, W], F32, tag="pu", padded_shape=[128, 512])
                for k in range(KT):
                    nc.tensor.matmul(
                        pus,
                        lhsT=su_sb[:, k * SM:(k + 1) * SM],
                        rhs=xt_t[:, k * W:(k + 1) * W],
                        start=(k == 0),
                        stop=(k == KT - 1),
                    )
                sig_s = sb.tile([128, W], BF16, tag="sig", padded_shape=[128, 512])
                nc.scalar.activation(sig_s, pgs, ACT.Sigmoid)
                sgs = sb.tile([128, W], BF16, tag="silu", padded_shape=[128, 512])
                nc.vector.scalar_tensor_tensor(
                    sgs, pgs, 1.0, sig_s, op0=OP.mult, op1=OP.mult
                )
                inter_s = sb.tile([128, W], BF16, tag="inter_s",
                                  padded_shape=[128, 512])
                nc.vector.tensor_mul(inter_s, sgs, pus)
                return inter_s

            def emit_down_and_rs(c, inters, inter_s):
                W = CHW[c]
                off = CHOFF[c]
                ypart = dram.tile([H, W], F32, name=f"ypart{c}", tag=f"ypart{c}")
                for ht in range(HT):
                    py = ps.tile([128, W], F32, tag="py", padded_shape=[128, 512])
                    first = True
                    for e in range(EPC):
                        for mb in range(MB):
                            nc.tensor.matmul(
                                py,
                                lhsT=wd_sb[e][:, mb * H + ht * 128: mb * H + (ht + 1) * 128],
                                rhs=inters[e][:, mb * W:(mb + 1) * W],
                                start=first,
                                stop=False,
                            )
                            first = False
                    nc.tensor.matmul(
                        py,
                        lhsT=sd_sb[:, ht * 128:(ht + 1) * 128],
                        rhs=inter_s,
                        start=False,
                        stop=True,
                    )
                    yp = sb.tile([128, W], F32, tag="yp", padded_shape=[128, 512])
                    nc.vector.tensor_copy(yp, py)
                    nc.sync.dma_start(
                        out=ypart[ht * 128:(ht + 1) * 128, :], in_=yp
                    )
                rs_out = dram.tile([128, W], F32, name=f"rsout{c}", tag=f"rsout{c}")
                nc.gpsimd.collective_compute(
                    "ReduceScatter",
                    OP.add,
                    replica_groups=[list(range(NCORES))],
                    ins=[ypart.opt()],
                    outs=[rs_out.opt()],
                )
                nc.sync.dma_start(out=out[:, off:off + W], in_=rs_out)

            # ---- software-pipelined main loop (gating one chunk ahead) ----
            cbc = {0: emit_gating_pe_tail(0, emit_gating(0, xts[0][1]))}
            xts[1] = emit_xt_dma(1)

            for c in range(NCH):
                xt_t = xts[c][0]
                i0 = emit_expert_gu(c, 0, xt_t, cbc[c][0])
                combs_next = None
                if c + 1 < NCH:
                    combs_next = emit_gating(c + 1, xts[c + 1][1])
                i1 = emit_expert_gu(c, 1, xt_t, cbc[c][1])
                inter_s = emit_shared(c, xt_t)
                if c + 1 < NCH:
                    cbc[c + 1] = emit_gating_pe_tail(c + 1, combs_next)
                if c + 2 < NCH:
                    xts[c + 2] = emit_xt_dma(c + 2)
                emit_down_and_rs(c, [i0, i1], inter_s)

    nc.compile()
    return nc


_NC_CACHE = None


def _get_program():
    global _NC_CACHE
    if _NC_CACHE is None:
        _NC_CACHE = build_program()
    return _NC_CACHE


def _make_in_maps(inputs):
    import ml_dtypes
    bf16 = ml_dtypes.bfloat16
    x = np.ascontiguousarray(
        np.asarray(inputs["hidden_states"], dtype=np.float32).reshape(N, H).T
    )
    x_bf = x.astype(bf16)
    gk = np.ascontiguousarray(np.asarray(inputs["gate_kernel"], dtype=np.float32))
    gb = np.asarray(inputs["gate_bias"], dtype=np.float32)
    gbr = np.ascontiguousarray(np.broadcast_to(gb[None, :], (128, E)))
    w_gate = np.asarray(inputs["w_gate"], dtype=np.float32)
    w_up = np.asarray(inputs["w_up"], dtype=np.float32)
    w_down = np.asarray(inputs["w_down"], dtype=np.float32)
    sw_gate = np.asarray(inputs["sw_gate"], dtype=np.float32)
    sw_up = np.asarray(inputs["sw_up"], dtype=np.float32)
    sw_down = np.asarray(inputs["sw_down"], dtype=np.float32)

    in_maps = []
    for c in range(NCORES):
        sel = np.zeros((EPC, E, 128), dtype=np.float32)
        for e in range(EPC):
            sel[e, EPC * c + e, :] = 1.0
        in_maps.append({
            "xT": x_bf,
            "xTf": x,
            "gk": gk,
            "gbr": gbr,
            "wg": np.ascontiguousarray(w_gate[EPC * c:EPC * (c + 1)]).astype(bf16),
            "wu": np.ascontiguousarray(w_up[EPC * c:EPC * (c + 1)]).astype(bf16),
            "wd": np.ascontiguousarray(w_down[EPC * c:EPC * (c + 1)]).astype(bf16),
            "sg": np.ascontiguousarray(sw_gate[:, SM * c:SM * (c + 1)]).astype(bf16),
            "su": np.ascontiguousarray(sw_up[:, SM * c:SM * (c + 1)]).astype(bf16),
            "sd": np.ascontiguousarray(sw_down[SM * c:SM * (c + 1), :]).astype(bf16),
            "sel": sel,
        })
    return in_maps


def run(inputs, trace=False):
    """Returns (output, BassKernelResults)."""
    nc = _get_program()
    in_maps = _make_in_maps(inputs)
    res = run_bass_kernel_spmd(
        nc, in_maps, core_ids=list(range(NCORES)), trace=trace
    )
    yT = np.concatenate(
        [res.results[c]["out"] for c in range(NCORES)], axis=0
    )
    y = np.ascontiguousarray(yT.T).reshape(2, 1024, H).astype(np.float32)
    return y, res


def kernel(**inputs):
    y, _ = run(inputs, trace=False)
    return y


# revision 10
# speedup vs baseline: 1.3448x; 1.0482x over previous
"""DeepseekV3 MoE kernel for 8 Trainium2 NeuronCores.

Sharding: expert-parallel (2 routed experts per core) + intermediate-sharded
shared expert (128 of 1024 columns per core); gate replicated (computed in
full fp32 on every core, pipelined one chunk ahead of the expert compute);
per-chunk ReduceScatter combines partial outputs; host concatenates shards.

Self-contained: hardcodes all shapes. Only dependency is the concourse
tree (on PYTHONPATH in the container) and numpy.
"""

import os
import sys

import numpy as np

for _p in ("/opt/trn_rl_repo", "/root/.axon_site/_ro/trn_rl_repo"):
    if os.path.isdir(_p) and _p not in sys.path:
        sys.path.append(_p)

import concourse.bacc as bacc
import concourse.mybir as mybir
import concourse.tile as tile
from concourse.bass_utils import run_bass_kernel_spmd
from concourse.masks import make_identity

F32 = mybir.dt.float32
F32R = mybir.dt.float32r
BF16 = mybir.dt.bfloat16
AX = mybir.AxisListType.X
OP = mybir.AluOpType
ACT = mybir.ActivationFunctionType

H = 1024          # hidden size
M = 512           # expert intermediate
E = 16            # routed experts
EPC = 2           # experts per core
NCORES = 8
N = 2048          # tokens (B*S)
KT = H // 128     # 8 contraction tiles
MB = M // 128     # 4 m-tiles per expert
HT = H // 128     # 8 output h-tiles
SCALE = 2.5
SM = 128          # shared-expert intermediate columns per core

# token chunks: smaller final chunks shrink the un-overlapped tail
# (last ReduceScatter + output DMA)
CHW = [512, 512, 512, 256, 256]
CHOFF = [0, 512, 1024, 1536, 1792]
NCH = len(CHW)


def _routing(nc, pool, s4c, comb):
    """Token-major DeepseekV3 noaux_tc routing for one [128, 16] tile.

    s4c: sigmoid(logits) + bias, [128, 16] fp32 SBUF.
    comb: output combine weights [128, 16] (SCALE * topk_weight scattered).
    """
    v = s4c.rearrange("p (g s) -> p g s", g=4)

    # sum of top-2 per group of 4 = max over the 6 pairwise sums
    pairs = pool.tile([128, 24], F32, tag="rt_pairs")
    pv = pairs.rearrange("p (g s) -> p g s", g=4)
    nc.vector.tensor_add(pv[:, :, 0:3], v[:, :, 0:3], v[:, :, 1:4])
    nc.vector.tensor_add(pv[:, :, 3:5], v[:, :, 0:2], v[:, :, 2:4])
    nc.vector.tensor_add(pv[:, :, 5:6], v[:, :, 0:1], v[:, :, 3:4])
    gsum = pool.tile([128, 4], F32, tag="rt_gsum")
    nc.vector.reduce_max(out=gsum, in_=pv, axis=AX)

    # 2nd largest group sum = max over the 6 pairwise mins
    gmins = pool.tile([128, 8], F32, tag="rt_gmins")
    nc.vector.tensor_tensor(gmins[:, 0:3], gsum[:, 0:3], gsum[:, 1:4], op=OP.min)
    nc.vector.tensor_tensor(gmins[:, 3:5], gsum[:, 0:2], gsum[:, 2:4], op=OP.min)
    nc.vector.tensor_tensor(gmins[:, 5:6], gsum[:, 0:1], gsum[:, 3:4], op=OP.min)
    t2g = pool.tile([128, 1], F32, tag="rt_t2g")
    nc.vector.reduce_max(out=t2g, in_=gmins[:, 0:6], axis=AX)

    # group mask (1.0 for the top-2 groups), expanded to 16 experts
    gmask = pool.tile([128, 4], F32, tag="rt_gmask")
    nc.vector.tensor_scalar(gmask, gsum, t2g, None, op0=OP.is_ge)
    mask16 = pool.tile([128, 16], F32, tag="rt_mask16")
    m16v = mask16.rearrange("p (g s) -> p g s", g=4)
    for j in range(4):
        nc.vector.tensor_copy(m16v[:, :, j], gmask)

    masked = pool.tile([128, 16], F32, tag="rt_masked")
    nc.vector.tensor_mul(masked, s4c, mask16)

    # top-4 of 16 via Max8, threshold select, normalize
    top8 = pool.tile([128, 8], F32, tag="rt_top8")
    nc.vector.max(out=top8, in_=masked)
    denom = pool.tile([128, 1], F32, tag="rt_denom")
    nc.vector.reduce_sum(out=denom, in_=top8[:, 0:4], axis=AX)
    w = pool.tile([128, 1], F32, tag="rt_w")
    nc.vector.tensor_scalar_add(denom, denom, 1e-20)
    nc.vector.reciprocal(w, denom)
    nc.vector.tensor_scalar_mul(w, w, SCALE)

    # sel_w = (masked >= t4) * w ; comb = sel_w * masked
    selw = pool.tile([128, 16], F32, tag="rt_selw")
    nc.vector.tensor_scalar(selw, masked, top8[:, 3:4], w, op0=OP.is_ge, op1=OP.mult)
    nc.vector.tensor_mul(comb, selw, masked)


def build_program():
    nc = bacc.Bacc(
        "TRN2",
        target_bir_lowering=False,
        debug=False,
        enable_asserts=False,
        num_devices=NCORES,
    )

    xT = nc.dram_tensor("xT", [H, N], BF16, kind="ExternalInput").ap()
    xTf = nc.dram_tensor("xTf", [H, N], F32, kind="ExternalInput").ap()
    gk = nc.dram_tensor("gk", [H, E], F32, kind="ExternalInput").ap()
    gbr = nc.dram_tensor("gbr", [128, E], F32, kind="ExternalInput").ap()
    wg = nc.dram_tensor("wg", [EPC, H, M], BF16, kind="ExternalInput").ap()
    wu = nc.dram_tensor("wu", [EPC, H, M], BF16, kind="ExternalInput").ap()
    wd = nc.dram_tensor("wd", [EPC, M, H], BF16, kind="ExternalInput").ap()
    sg = nc.dram_tensor("sg", [H, SM], BF16, kind="ExternalInput").ap()
    su = nc.dram_tensor("su", [H, SM], BF16, kind="ExternalInput").ap()
    sd = nc.dram_tensor("sd", [SM, H], BF16, kind="ExternalInput").ap()
    sel_in = nc.dram_tensor("sel", [EPC, E, 128], F32R, kind="ExternalInput").ap()
    out = nc.dram_tensor("out", [128, N], F32, kind="ExternalOutput").ap()

    with tile.TileContext(nc) as tc:
        with (
            tc.tile_pool(name="w", bufs=1) as wpool,
            tc.tile_pool(name="sb", bufs=2) as sb,
            tc.tile_pool(name="rt", bufs=2) as rt,
            tc.tile_pool(name="ps", bufs=2, space="PSUM") as ps,
            tc.tile_pool(name="dram", bufs=1, space="DRAM") as dram,
        ):
            # ---- gating-critical small DMAs first ----
            gk_sb = wpool.tile([128, KT * E], F32, tag="gk")
            for k in range(KT):
                nc.sync.dma_start(
                    out=gk_sb[:, k * E:(k + 1) * E],
                    in_=gk[k * 128:(k + 1) * 128, :],
                )
            gbr_sb = wpool.tile([128, E], F32, tag="gbr")
            nc.sync.dma_start(out=gbr_sb, in_=gbr)
            selm_sb = wpool.tile([E, EPC * 128], F32R, tag="selm")
            for e in range(EPC):
                nc.sync.dma_start(
                    out=selm_sb[:, e * 128:(e + 1) * 128], in_=sel_in[e]
                )
            ident = wpool.tile([128, 128], F32, tag="ident")
            make_identity(nc, ident)

            def emit_xt_dma(c):
                W = CHW[c]
                off = CHOFF[c]
                xt_t = sb.tile([128, KT * W], BF16, tag="xt", bufs=3,
                               padded_shape=[128, KT * 512])
                xtf_t = sb.tile([128, KT * W], F32, tag="xtf", bufs=2,
                                padded_shape=[128, KT * 512])
                for k in range(KT):
                    nc.sync.dma_start(
                        out=xtf_t[:, k * W:(k + 1) * W],
                        in_=xTf[k * 128:(k + 1) * 128, off:off + W],
                    )
                for k in range(KT):
                    nc.sync.dma_start(
                        out=xt_t[:, k * W:(k + 1) * W],
                        in_=xT[k * 128:(k + 1) * 128, off:off + W],
                    )
                return xt_t, xtf_t

            xts = {0: emit_xt_dma(0)}

            # ---- resident weights (after chunk-0 activations) ----
            wg_sb = []
            wu_sb = []
            wd_sb = []
            for e in range(EPC):
                g_t = wpool.tile([128, KT * M], BF16, name=f"wg_sb{e}", tag=f"wg{e}")
                u_t = wpool.tile([128, KT * M], BF16, name=f"wu_sb{e}", tag=f"wu{e}")
                for k in range(KT):
                    nc.sync.dma_start(
                        out=g_t[:, k * M:(k + 1) * M],
                        in_=wg[e, k * 128:(k + 1) * 128, :],
                    )
                    nc.sync.dma_start(
                        out=u_t[:, k * M:(k + 1) * M],
                        in_=wu[e, k * 128:(k + 1) * 128, :],
                    )
                wg_sb.append(g_t)
                wu_sb.append(u_t)

            sg_sb = wpool.tile([128, KT * SM], BF16, tag="sg")
            su_sb = wpool.tile([128, KT * SM], BF16, tag="su")
            for k in range(KT):
                nc.sync.dma_start(
                    out=sg_sb[:, k * SM:(k + 1) * SM],
                    in_=sg[k * 128:(k + 1) * 128, :],
                )
                nc.sync.dma_start(
                    out=su_sb[:, k * SM:(k + 1) * SM],
                    in_=su[k * 128:(k + 1) * 128, :],
                )

            for e in range(EPC):
                d_t = wpool.tile([128, MB * H], BF16, name=f"wd_sb{e}", tag=f"wd{e}")
                for mb in range(MB):
                    nc.sync.dma_start(
                        out=d_t[:, mb * H:(mb + 1) * H],
                        in_=wd[e, mb * 128:(mb + 1) * 128, :],
                    )
                wd_sb.append(d_t)
            sd_sb = wpool.tile([128, H], BF16, tag="sd")
            nc.sync.dma_start(out=sd_sb, in_=sd)

            def emit_gating(c, xtf_t):
                """fp32 token-major logits + routing; returns comb tiles."""
                W = CHW[c]
                combs = []
                for t in range(W // 128):
                    plt = ps.tile([128, E], F32, tag="pmisc")
                    for k in range(KT):
                        nc.tensor.matmul(
                            plt,
                            lhsT=xtf_t[:, k * W + t * 128: k * W + (t + 1) * 128],
                            rhs=gk_sb[:, k * E:(k + 1) * E],
                            start=(k == 0),
                            stop=(k == KT - 1),
                        )
                    s4c = rt.tile([128, E], F32, tag="rt_s4c")
                    nc.scalar.activation(s4c, plt, ACT.Sigmoid)
                    nc.vector.tensor_add(s4c, s4c, gbr_sb)
                    comb = rt.tile([128, E], F32, tag="rt_comb", bufs=8)
                    _routing(nc, rt, s4c, comb)
                    combs.append(comb)
                return combs

            def emit_gating_pe_tail(c, combs):
                """transpose combine + broadcast local experts' rows."""
                W = CHW[c]
                combT = sb.tile([E, W], F32R, tag="combT",
                                padded_shape=[E, 512])
                for t, comb in enumerate(combs):
                    pct = ps.tile([E, 128], F32, tag="pmisc")
                    nc.tensor.transpose(pct, comb, ident)
                    nc.scalar.copy(combT[:, t * 128:(t + 1) * 128], pct)
                cbc = []
                for e in range(EPC):
                    pb = ps.tile([128, W], F32, tag="pmisc",
                                 padded_shape=[128, 512])
                    nc.tensor.matmul(
                        pb,
                        lhsT=selm_sb[:, e * 128:(e + 1) * 128],
                        rhs=combT,
                        start=True,
                        stop=True,
                    )
                    cb = sb.tile([128, W], F32, tag=f"cbc{e}",
                                 padded_shape=[128, 512])
                    nc.scalar.copy(cb, pb)
                    cbc.append(cb)
                return cbc

            def emit_expert_gu(c, e, xt_t, cbc_e):
                """g/u projections + inter = silu(g) * u * combine."""
                W = CHW[c]
                it = sb.tile([128, MB * W], BF16, tag=f"inter{e}", bufs=1,
                             padded_shape=[128, MB * 512])
                for mb in range(MB):
                    pg = ps.tile([128, W], F32, tag="pg", padded_shape=[128, 512])
                    for k in range(KT):
                        nc.tensor.matmul(
                            pg,
                            lhsT=wg_sb[e][:, k * M + mb * 128: k * M + (mb + 1) * 128],
                            rhs=xt_t[:, k * W:(k + 1) * W],
                            start=(k == 0),
                            stop=(k == KT - 1),
                        )
                    pu = ps.tile([128, W], F32, tag="pu", padded_shape=[128, 512])
                    for k in range(KT):
                        nc.tensor.matmul(
                            pu,
                            lhsT=wu_sb[e][:, k * M + mb * 128: k * M + (mb + 1) * 128],
                            rhs=xt_t[:, k * W:(k + 1) * W],
                            start=(k == 0),
                            stop=(k == KT - 1),
                        )
                    sig_t = sb.tile([128, W], BF16, tag="sig",
                                    padded_shape=[128, 512])
                    nc.scalar.activation(sig_t, pg, ACT.Sigmoid)
                    sg_t = sb.tile([128, W], BF16, tag="silu",
                                   padded_shape=[128, 512])
                    nc.vector.scalar_tensor_tensor(
                        sg_t, pg, 1.0, sig_t, op0=OP.mult, op1=OP.mult
                    )
                    us = sb.tile([128, W], BF16, tag="us", padded_shape=[128, 512])
                    nc.vector.tensor_mul(us, pu, cbc_e)
                    nc.vector.tensor_mul(it[:, mb * W:(mb + 1) * W], sg_t, us)
                return it

            def emit_shared(c, xt_t):
                W = CHW[c]
                pgs = ps.tile([128, W], F32, tag="pg", padded_shape=[128, 512])
                for k in range(KT):
                    nc.tensor.matmul(
                        pgs,
                        lhsT=sg_sb[:, k * SM:(k + 1) * SM],
                        rhs=xt_t[:, k * W:(k + 1) * W],
                        start=(k == 0),
                        stop=(k == KT - 1),
                    )
                pus = ps.tile([128, W], F32, tag="pu", padded_shape=[128, 512])
                for k in range(KT):
                    nc.tensor.matmul(
                        pus,
                        lhsT=su_sb[:, k * SM:(k + 1) * SM],
                        rhs=xt_t[:, k * W:(k + 1) * W],
                        start=(k == 0),
                        stop=(k == KT - 1),
                    )
                sig_s = sb.tile([128, W], BF16, tag="sig", padded_shape=[128, 512])
                nc.scalar.activation(sig_s, pgs, ACT.Sigmoid)
                sgs = sb.tile([128, W], BF16, tag="silu", padded_shape=[128, 512])
                nc.vector.scalar_tensor_tensor(
                    sgs, pgs, 1.0, sig_s, op0=OP.mult, op1=OP.mult
                )
                inter_s = sb.tile([128, W], BF16, tag="inter_s",
                                  padded_shape=[128, 512])
                nc.vector.tensor_mul(inter_s, sgs, pus)
                return inter_s

            def emit_down_and_rs(c, inters, inter_s):
                W = CHW[c]
                off = CHOFF[c]
                ypart = dram.tile([H, W], F32, name=f"ypart{c}", tag=f"ypart{c}")
                for ht in range(HT):
                    py = ps.tile([128, W], F32, tag="py", padded_shape=[128, 512])
                    first = True
                    for e in range(EPC):
                        for mb in range(MB):
                            nc.tensor.matmul(
                                py,
                                lhsT=wd_sb[e][:, mb * H + ht * 128: mb * H + (ht + 1) * 128],
                                rhs=inters[e][:, mb * W:(mb + 1) * W],
                                start=first,
                                stop=False,
                            )
                            first = False
                    nc.tensor.matmul(
                        py,
                        lhsT=sd_sb[:, ht * 128:(ht + 1) * 128],
                        rhs=inter_s,
                        start=False,
                        stop=True,
                    )
                    yp = sb.tile([128, W], F32, tag="yp", padded_shape=[128, 512])
                    nc.vector.tensor_copy(yp, py)
                    nc.sync.dma_start(
                        out=ypart[ht * 128:(ht + 1) * 128, :], in_=yp
                    )
                rs_out = dram.tile([128, W], F32, name=f"rsout{c}", tag=f"rsout{c}")
                nc.gpsimd.collective_compute(
                    "ReduceScatter",
                    OP.add,
                    replica_groups=[list(range(NCORES))],
                    ins=[ypart.opt()],
                    outs=[rs_out.opt()],
                )
                nc.gpsimd.dma_start(out=out[:, off:off + W], in_=rs_out)

            # ---- software-pipelined main loop (gating one chunk ahead) ----
            cbc = {0: emit_gating_pe_tail(0, emit_gating(0, xts[0][1]))}
            xts[1] = emit_xt_dma(1)

            for c in range(NCH):
                xt_t = xts[c][0]
                i0 = emit_expert_gu(c, 0, xt_t, cbc[c][0])
                combs_next = None
                if c + 1 < NCH:
                    combs_next = emit_gating(c + 1, xts[c + 1][1])
                i1 = emit_expert_gu(c, 1, xt_t, cbc[c][1])
                inter_s = emit_shared(c, xt_t)
                if c + 1 < NCH:
                    cbc[c + 1] = emit_gating_pe_tail(c + 1, combs_next)
                if c + 2 < NCH:
                    xts[c + 2] = emit_xt_dma(c + 2)
                emit_down_and_rs(c, [i0, i1], inter_s)

    nc.compile()
    return nc


_NC_CACHE = None


def _get_program():
    global _NC_CACHE
    if _NC_CACHE is None:
        _NC_CACHE = build_program()
    return _NC_CACHE


def _make_in_maps(inputs):
    import ml_dtypes
    bf16 = ml_dtypes.bfloat16
    x = np.ascontiguousarray(
        np.asarray(inputs["hidden_states"], dtype=np.float32).reshape(N, H).T
    )
    x_bf = x.astype(bf16)
    gk = np.ascontiguousarray(np.asarray(inputs["gate_kernel"], dtype=np.float32))
    gb = np.asarray(inputs["gate_bias"], dtype=np.float32)
    gbr = np.ascontiguousarray(np.broadcast_to(gb[None, :], (128, E)))
    w_gate = np.asarray(inputs["w_gate"], dtype=np.float32)
    w_up = np.asarray(inputs["w_up"], dtype=np.float32)
    w_down = np.asarray(inputs["w_down"], dtype=np.float32)
    sw_gate = np.asarray(inputs["sw_gate"], dtype=np.float32)
    sw_up = np.asarray(inputs["sw_up"], dtype=np.float32)
    sw_down = np.asarray(inputs["sw_down"], dtype=np.float32)

    in_maps = []
    for c in range(NCORES):
        sel = np.zeros((EPC, E, 128), dtype=np.float32)
        for e in range(EPC):
            sel[e, EPC * c + e, :] = 1.0
        in_maps.append({
            "xT": x_bf,
            "xTf": x,
            "gk": gk,
            "gbr": gbr,
            "wg": np.ascontiguousarray(w_gate[EPC * c:EPC * (c + 1)]).astype(bf16),
            "wu": np.ascontiguousarray(w_up[EPC * c:EPC * (c + 1)]).astype(bf16),
            "wd": np.ascontiguousarray(w_down[EPC * c:EPC * (c + 1)]).astype(bf16),
            "sg": np.ascontiguousarray(sw_gate[:, SM * c:SM * (c + 1)]).astype(bf16),
            "su": np.ascontiguousarray(sw_up[:, SM * c:SM * (c + 1)]).astype(bf16),
            "sd": np.ascontiguousarray(sw_down[SM * c:SM * (c + 1), :]).astype(bf16),
            "sel": sel,
        })
    return in_maps


def run(inputs, trace=False):
    """Returns (output, BassKernelResults)."""
    nc = _get_program()
    in_maps = _make_in_maps(inputs)
    res = run_bass_kernel_spmd(
        nc, in_maps, core_ids=list(range(NCORES)), trace=trace
    )
    yT = np.concatenate(
        [res.results[c]["out"] for c in range(NCORES)], axis=0
    )
    y = np.ascontiguousarray(yT.T).reshape(2, 1024, H).astype(np.float32)
    return y, res


def kernel(**inputs):
    y, _ = run(inputs, trace=False)
    return y
